# revision 40
# baseline (speedup 1.0000x reference)
"""Trainium2 Bass kernel: prototype-kNN CCE loss (nn_CCE_67190468378875).

Math: for each row b, d2[b,j] = |x_b|^2 + |w_j|^2 - 2 x_b.w_j over CP=6400
prototypes (200 classes x 32 protos).  The loss only needs, per row, the
min-over-protos-per-class distance at the target class (v_t) and the min over
all other classes (v_w); the gathered-prototype MSEs in the reference equal
exactly those squared distances averaged over rows (and /F).

Device work per core (batch-sharded 512 rows):
  nq[b,j] = 2 x_b.w_j - |w_j|^2   (matmul of bf16 X^T against bf16 (2W)^T,
                                   p2 folded in with a DVE add)
  per-class max of nq -> max_nq[b,c]  (= -min d2 + |x_b|^2 term deferred)
  v_t[b] = -max_nq[b, tc_b],  v_w[b] = -max over c != tc_b  (tensor_mask_reduce)
Host: sums, add sum|x|^2, final scalar combine.
"""

import numpy as np
import ml_dtypes
from contextlib import ExitStack

import concourse.bass as bass
import concourse.mybir as mybir
import concourse.tile as tile
from concourse.bass_utils import run_bass_kernel_spmd

B, C, P, F = 4096, 200, 32, 512
CP = C * P                  # 6400 prototypes
ALPHA, EPS = 5.0, 1e-8
N_CORES = 8
BLOC = B // N_CORES         # 512 rows per core
BB = BLOC // 128            # 4 row-blocks of 128
FC = F // 128               # 4 contraction chunks
JSB = 1024                  # prototype super-block (2 PSUM banks)
NJSB = (CP + JSB - 1) // JSB

_BF16 = mybir.dt.bfloat16
_F32 = mybir.dt.float32

# Matmul operand precision: "fp8" (e4m3 + DoubleRow, ~1.5x PE) or "bf16".
# Final-loss rel err measured on this input: fp8 ~1.0e-3, bf16 ~4e-5.
MM_MODE = "fp8"


def _emit(ctx, tc_ctx, io):
    nc = tc_ctx.nc
    singles = ctx.enter_context(tc_ctx.tile_pool(name="singles", bufs=1))
    psum = ctx.enter_context(tc_ctx.tile_pool(name="psum", bufs=3, space="PSUM"))
    dps = ctx.enter_context(tc_ctx.tile_pool(name="dps", bufs=1, space="PSUM"))
    scr = ctx.enter_context(tc_ctx.tile_pool(name="scr", bufs=2))

    mm_dt = mybir.dt.float8e4 if MM_MODE == "fp8" else _BF16
    wt_t = singles.tile([128, FC, CP], mm_dt)    # (2W)^T  [f, j]
    xt_t = singles.tile([128, FC, BLOC], mm_dt)  # X^T     [f, b]
    p2_t = singles.tile([1, CP], _BF16)          # -|w|^2 row (K=1 fold rhs)
    ones_t = singles.tile([1, 128], _BF16)       # K=1 fold lhsT
    maskt_t = singles.tile([128, BB, C], _BF16)  # BIG where c != target
    maskw_t = singles.tile([128, BB, C], _BF16)  # BIG where c == target
    minq = singles.tile([128, BB, C], _F32)      # per-class max of nq
    vt_t = singles.tile([128, BB], _F32)         # negated v_t
    vw_t = singles.tile([128, BB], _F32)         # negated v_w

    # Dead PSUM bank for sacrificial 1-column matmuls: each input DMA is
    # "observed" by PE through one of these, so no real matmul ever needs
    # more than 2 inline sync waits (walrus wait-slot limits per opcode).
    dummy_ps = dps.tile([1, 1], _F32)

    def pe_observe(sb_col):
        return nc.tensor.matmul(dummy_ps[:1, :1], sb_col, sb_col,
                                start=True, stop=True)

    nc.sync.dma_start(out=maskt_t[:, :, :], in_=io["maskt"][:, :, :])
    nc.sync.dma_start(out=maskw_t[:, :, :], in_=io["maskw"][:, :, :])
    for fc in range(FC):
        nc.sync.dma_start(out=xt_t[:, fc, :], in_=io["xt"][fc * 128:(fc + 1) * 128, :])
        pe_observe(xt_t[:, fc, 0:1])
    nc.sync.dma_start(out=p2_t[0:1, :], in_=io["p2n"][:, :])
    nc.vector.memset(ones_t[0:1, :], 1.0)
    pe_observe(p2_t[0:1, 0:1])
    pe_observe(ones_t[0:1, 0:1])
    # W^T loads, j-major so the first super-blocks land first
    for j in range(NJSB):
        j0 = j * JSB
        w = min(JSB, CP - j0)
        for fc in range(FC):
            nc.sync.dma_start(out=wt_t[:, fc, j0:j0 + w],
                              in_=io["wt"][fc * 128:(fc + 1) * 128, j0:j0 + w])
            pe_observe(wt_t[:, fc, j0:j0 + 1])

    minq_hist = []
    grp = 0
    for j in range(NJSB):
        j0 = j * JSB
        w = min(JSB, CP - j0)
        ncls = w // P
        for bb in range(BB):
            guard = None
            if grp >= 3:
                # PE observes the DVE reduce that freed the psum bank this
                # group reuses (bufs=3), absorbing the DVE wait off the
                # group's first real matmul.
                guard = pe_observe(minq_hist[grp - 3])
            grp += 1
            ps = psum.tile([128, JSB], _F32, tag="ps")
            fstep = 2 if MM_MODE == "fp8" else 1
            pmode = (mybir.MatmulPerfMode.DoubleRow if MM_MODE == "fp8"
                     else None)
            for h0 in range(0, w, 512):
                hw = min(512, w - h0)
                for fc in range(0, FC, fstep):
                    if fstep == 2:
                        lhs = xt_t[:, fc:fc + 2, bb * 128:(bb + 1) * 128]
                        rhs = wt_t[:, fc:fc + 2, j0 + h0:j0 + h0 + hw]
                    else:
                        lhs = xt_t[:, fc, bb * 128:(bb + 1) * 128]
                        rhs = wt_t[:, fc, j0 + h0:j0 + h0 + hw]
                    mm = nc.tensor.matmul(ps[:, h0:h0 + hw], lhs, rhs,
                                          start=(fc == 0), stop=False,
                                          perf_mode=pmode)
                    if guard is not None:
                        tile.add_dep_helper(mm.ins, guard.ins,
                                            reason="keep bank guard first")
                        guard = None
                # fold -|w|^2 into the accumulation: rank-1 ones (x) p2 row
                nc.tensor.matmul(ps[:, h0:h0 + hw], ones_t[0:1, :],
                                 p2_t[0:1, j0 + h0:j0 + h0 + hw],
                                 start=False, stop=True)
            out_sl = minq[:, bb, j0 // P: j0 // P + ncls]
            minq_hist.append(out_sl[:, 0:1])
            nc.vector.tensor_reduce(
                out=out_sl,
                in_=ps[:, :w].rearrange("p (c q) -> p c q", q=P),
                axis=mybir.AxisListType.X, op=mybir.AluOpType.max)

    # Selection: vt_t[p,bb] = max_c (minq - BIG*(c != tc)), i.e. minq at the
    # target class; vw_t = max over the other classes. Host negates.
    for mask, acc in ((maskt_t, vt_t), (maskw_t, vw_t)):
        sel = scr.tile([128, BB, C], _F32, tag="sel")
        nc.vector.tensor_sub(sel[:, :, :], minq[:, :, :], mask[:, :, :])
        nc.vector.tensor_reduce(out=acc[:, :], in_=sel[:, :, :],
                                axis=mybir.AxisListType.X,
                                op=mybir.AluOpType.max)
    nc.sync.dma_start(out=io["vt"][:, :], in_=vt_t[:, :])
    nc.sync.dma_start(out=io["vw"][:, :], in_=vw_t[:, :])


_RANGE_CLEAR_OPCODE = 176


def _legalize_sync(nc):
    """Adapt the Tile-scheduled module to this container's walrus build:

    1. TPB instruction encodings here accept at most ONE inline sync wait
       ("Too many sync wait commands"), so hoist extra waits into standalone
       single-wait EventSemaphore instructions on the same engine.
    2. The tail EVENT_SEMAPHORE_RANGE_CLEAR InstISA is rejected ("ISA wrong
       length"); replace it with per-semaphore write-0 updates.
    """
    wid = [0]

    def mk(engine, waits, updates):
        ev = mybir.InstEventSemaphore(name=f"WSPLIT-{wid[0]}")
        wid[0] += 1
        ev.engine = engine
        ev.sync_info = mybir.SyncInfo(on_wait=waits, on_update=updates)
        return ev

    for fn in nc.m.functions:
        for blk in fn.blocks:
            out = []
            for ins in blk.instructions:
                si = ins.sync_info
                if si is not None and len(si.on_wait) > 1:
                    for w in si.on_wait[:-1]:
                        out.append(mk(ins.engine, [w], []))
                    ins.sync_info = mybir.SyncInfo(
                        on_wait=[si.on_wait[-1]], on_update=list(si.on_update))
                if (type(ins).__name__ == "InstDrain"
                        and getattr(ins, "is_reset_sema", False)):
                    first = ins.reset_range_start
                    last = ins.reset_range_stop - 1
                    ins.is_reset_sema = False
                    ups = [mybir.SyncUpdate(sync_type="semaphore", id=s,
                                            update_mode="sem-wr-imm",
                                            update_value=0)
                           for s in range(first, last + 1)]
                    out.append(ins)
                    for u in ups:
                        out.append(mk(ins.engine, [], [u]))
                    continue
                if (type(ins).__name__ == "InstISA"
                        and getattr(ins, "isa_opcode", None) == _RANGE_CLEAR_OPCODE):
                    import re as _re
                    m = _re.search(r"range_first=(\d+) range_last=(\d+)", str(ins))
                    first, last = int(m.group(1)), int(m.group(2))
                    ups = [mybir.SyncUpdate(sync_type="semaphore", id=s,
                                            update_mode="sem-wr-imm",
                                            update_value=0)
                           for s in range(first, last + 1)]
                    for u in ups:
                        out.append(mk(ins.engine, [], [u]))
                    continue
                out.append(ins)
            blk.set_instructions(out) if hasattr(blk, "set_instructions") else None
            if not hasattr(blk, "set_instructions"):
                blk.instructions = out


_NC_CACHE = {}


def build_nc(legalize=True, reps=1, loop=0):
    key = (legalize, reps, loop)
    if key in _NC_CACHE:
        return _NC_CACHE[key]
    nc = bass.Bass()
    mm_dt = mybir.dt.float8e4 if MM_MODE == "fp8" else _BF16
    io = {
        "wt": nc.declare_dram_parameter("wt", [F, CP], mm_dt, isOutput=False),
        "xt": nc.declare_dram_parameter("xt", [F, BLOC], mm_dt, isOutput=False),
        "p2n": nc.declare_dram_parameter("p2n", [1, CP], _BF16, isOutput=False),
        "maskt": nc.declare_dram_parameter("maskt", [128, BB, C], _BF16,
                                           isOutput=False),
        "maskw": nc.declare_dram_parameter("maskw", [128, BB, C], _BF16,
                                           isOutput=False),
        "vt": nc.declare_dram_parameter("vt", [128, BB], _F32, isOutput=True),
        "vw": nc.declare_dram_parameter("vw", [128, BB], _F32, isOutput=True),
    }
    with tile.TileContext(nc) as tc_ctx:
        if loop:
            with tc_ctx.For_i(0, loop, 1):
                with ExitStack() as ctx:
                    _emit(ctx, tc_ctx, io)
        else:
            for _ in range(reps):
                with ExitStack() as ctx:
                    _emit(ctx, tc_ctx, io)
    if legalize:
        _legalize_sync(nc)
    _NC_CACHE[key] = nc
    return nc


def make_in_maps(outputs, clusters, target_classes):
    X = np.asarray(outputs, dtype=np.float32)
    W = np.asarray(clusters, dtype=np.float32).reshape(CP, F)
    tcl = np.asarray(target_classes).astype(np.int64)

    mm_np = ml_dtypes.float8_e4m3 if MM_MODE == "fp8" else ml_dtypes.bfloat16
    w2b = (2.0 * W).astype(mm_np)                         # [CP, F]
    wt = np.ascontiguousarray(w2b.T)                      # [F, CP]
    wf = w2b.astype(np.float32) * 0.5                     # the W the device sees
    p2n = (-np.sum(wf * wf, axis=1)).astype(ml_dtypes.bfloat16).reshape(1, CP)

    in_maps = []
    big = float(2 ** 30)
    for c in range(N_CORES):
        xs = X[c * BLOC:(c + 1) * BLOC]                   # [BLOC, F]
        xt = np.ascontiguousarray(xs.T.astype(mm_np))
        tc_pb = tcl[c * BLOC:(c + 1) * BLOC].reshape(BB, 128).T  # [128, BB]
        onehot = np.arange(C)[None, None, :] == tc_pb[:, :, None]
        in_maps.append({
            "wt": wt, "xt": xt, "p2n": p2n,
            "maskt": np.where(onehot, 0.0, big).astype(ml_dtypes.bfloat16),
            "maskw": np.where(onehot, big, 0.0).astype(ml_dtypes.bfloat16),
        })
    return in_maps, X


def combine(results, X):
    # Device outputs hold max_c(2x.w - |w|^2) at/off the target class; the
    # per-row squared distance contribution is the NEGATION of that.
    svt = -sum(float(r["vt"].astype(np.float64).sum()) for r in results)
    svw = -sum(float(r["vw"].astype(np.float64).sum()) for r in results)
    sx2 = float((X.astype(np.float64) ** 2).sum())
    tl = (sx2 + svt) / (B * F)
    ntl = (sx2 + svw) / (B * F)
    return np.float32((1.0 - ALPHA) * tl + ALPHA / (ntl + EPS))


def kernel(outputs, clusters, target_classes):
    nc = build_nc()
    in_maps, X = make_in_maps(outputs, clusters, target_classes)
    res = run_bass_kernel_spmd(nc, in_maps, core_ids=list(range(N_CORES))).results
    return combine(res, X)
